# revision 4
# baseline (speedup 1.0000x reference)
"""Deformable 3D trilinear sampling kernel for TRN2 (Bass/Tile).

Per core: NPAIRS (batch,group) pairs, each [C=8, T, H, W].
L staging layout (fp16, DRAM), padded to uniform rows:
  row r = tt*HB + hh  (HB = H+2 row slots per tt; hh in [0, H+1] live),
  each row = [ww(W+2), at(2), ah(2), c(C)] where
    L[tt, hh, ww, at, ah, c] = x[c, tt-1+at, hh-1+ah, ww-1]  (0 outside)
One contiguous 2*2*2*C-element run starting at 64B-unit index
  idx = (tt*HB + hh)*WW + ww   (ww = w0c+1, hh = h0c+1, tt = t0c+1)
holds all 8 trilinear corners x C channels of one sampling cell.

Host supplies x pre-transposed/padded as f16 [NP, T+2, H+3, W, C] so the
L build is: one DMA per slot piece (2 t-planes, w-major rows), then 4
unit-stride SBUF copies (at,ah shifts) into the interleaved mega tile.

Weight/index math is fused over quads of QC=4 chunks (one [128,3,QC,M]
tile per quantity); the per-quad t-origin is folded into the scalar
constants. Gather: ONE batched indirect DMA per chunk (128*M descriptors
of 128B). Corner combine: gpsimd multiply by broadcast W8, then an
in-place f16 tree reduction (ww, then at, then ah) on the vector engine.
"""

import dataclasses
import numpy as np

import concourse.bass as bass
import concourse.bacc as bacc
import concourse.mybir as mybir
import concourse.tile as tile
from concourse.bass import IndirectOffsetOnAxis

F32 = mybir.dt.float32
F16 = mybir.dt.float16
U32 = mybir.dt.uint32
AX = mybir.AxisListType
OP = mybir.AluOpType
ACT = mybir.ActivationFunctionType

MAGIC = 12582912.0  # 1.5 * 2**23: (x+M)-M = rne(x)


@dataclasses.dataclass
class Cfg:
    T: int = 8
    H: int = 128
    W: int = 128
    C: int = 8
    NPAIRS: int = 4
    M: int = 128
    QC: int = 4  # chunks fused per weight-math quad
    SCH: int = 126  # staging rows per mega-slot

    @property
    def N(self):
        return self.T * self.H * self.W

    @property
    def TT(self):
        return self.T + 1

    @property
    def HB(self):
        return self.H + 2

    @property
    def WW(self):
        return self.W + 2

    @property
    def NSLOTS(self):
        return (self.TT * self.HB + self.SCH - 1) // self.SCH

    @property
    def NR(self):
        return self.NSLOTS * self.SCH

    @property
    def ROWE(self):
        return self.WW * 4 * self.C

    @property
    def CHUNK(self):
        return 128 * self.M

    @property
    def NCHUNKS(self):
        assert self.N % self.CHUNK == 0
        return self.N // self.CHUNK

    @property
    def NQ(self):
        assert self.NCHUNKS % self.QC == 0
        return self.NCHUNKS // self.QC


def make_grids(cfg: Cfg) -> np.ndarray:
    t = np.arange(cfg.T, dtype=np.float32)[:, None, None]
    h = np.arange(cfg.H, dtype=np.float32)[None, :, None]
    w = np.arange(cfg.W, dtype=np.float32)[None, None, :]
    shape = (cfg.T, cfg.H, cfg.W)
    return np.stack(
        [np.broadcast_to(t, shape).ravel(),
         np.broadcast_to(h, shape).ravel(),
         np.broadcast_to(w, shape).ravel()]
    )


def build_program(cfg: Cfg, num_devices: int = 1, dbg: bool = False):
    nc = bacc.Bacc("TRN2", target_bir_lowering=False, debug=False,
                   enable_asserts=False, num_devices=num_devices)
    T, H, W, C, NP = cfg.T, cfg.H, cfg.W, cfg.C, cfg.NPAIRS
    TT, HB, WW, M, QC = cfg.TT, cfg.HB, cfg.WW, cfg.M, cfg.QC
    N, SCH, NSLOTS, NR, ROWE = cfg.N, cfg.SCH, cfg.NSLOTS, cfg.NR, cfg.ROWE
    CHUNK, NQ = cfg.CHUNK, cfg.NQ

    xs = nc.dram_tensor("xs", [NP, T + 2, H + 3, W, C], F16,
                        kind="ExternalInput").ap()
    offs = nc.dram_tensor("offs", [NP, 3, N], F32, kind="ExternalInput").ap()
    grids = nc.dram_tensor("grids", [3, QC * CHUNK], F32,
                           kind="ExternalInput").ap()
    outs = nc.dram_tensor("outs", [NP, C, N], F32, kind="ExternalOutput").ap()
    if dbg:
        dbg_idx = nc.dram_tensor("dbg_idx", [128, QC * M], U32,
                                 kind="ExternalOutput").ap()
        dbg_w8 = nc.dram_tensor("dbg_w8", [128, M * 8], F32,
                                kind="ExternalOutput").ap()
        dbg_g = nc.dram_tensor("dbg_g", [128, M * 64], F16,
                               kind="ExternalOutput").ap()
        dbg_l = nc.dram_tensor("dbg_l", [128, 512], F16,
                               kind="ExternalOutput").ap()

    Lt = [nc.dram_tensor(f"L{p}", [NR, ROWE], F16, kind="Internal").ap()
          for p in range(NP)]

    nlive = TT * HB

    def slot_pieces(r0):
        out = []
        r1 = min(r0 + SCH, nlive)
        for tt in range(TT):
            lo, hi = max(r0, tt * HB), min(r1, (tt + 1) * HB)
            if lo >= hi:
                continue
            out.append((lo - r0, hi - lo, tt, lo - tt * HB))
        return out

    with tile.TileContext(nc) as tc:
        with (
            tc.tile_pool(name="slot", bufs=3) as slotp,
            tc.tile_pool(name="stg", bufs=3) as stg,
            tc.tile_pool(name="gridp", bufs=1) as gridp,
            tc.tile_pool(name="wq", bufs=2) as wq,
            tc.tile_pool(name="wts", bufs=2) as wts,
            tc.tile_pool(name="gbig", bufs=2) as gbig,
            tc.tile_pool(name="oc", bufs=2) as ocp,
        ):
            g0 = gridp.tile([128, 3, QC, M], F32, tag="g0")
            for a in range(3):
                nc.sync.dma_start(
                    out=g0[:, a],
                    in_=grids[a].rearrange("(j p m) -> p j m", j=QC, m=M))
            for p in range(NP):
                # ---- build L slot-by-slot (small staging tiles; all copies
                # on the scalar engine so vector/gpsimd stay on the quads) ----
                for s in range(NSLOTS):
                    live = min(SCH, nlive - s * SCH)
                    st = slotp.tile([128, ROWE], F16, tag="st")
                    mv = st[:live].rearrange("p (w e) -> p w e", e=4 * C)
                    nc.vector.memset(mv[:, 0], 0.0)
                    nc.vector.memset(mv[:, WW - 1], 0.0)
                    cm = stg.tile([128, 2, 2, W * C], F16, tag="cm")
                    pieces = slot_pieces(s * SCH)
                    for at in range(2):
                        for ah in range(2):
                            for (p0, plen, tt, hh_lo) in pieces:
                                nc.sync.dma_start(
                                    out=cm[p0 : p0 + plen, at, ah],
                                    in_=xs[p][tt + at,
                                              hh_lo + ah : hh_lo + ah + plen]
                                    .rearrange("h w c -> h (w c)"),
                                )
                    for at in range(2):
                        for ah in range(2):
                            src = cm[:live, at, ah].rearrange(
                                "p (w c) -> p w c", c=C)
                            dst = st[:live].rearrange(
                                "p (w a b c) -> p w a b c", a=2, b=2, c=C
                            )[:, 1 : 1 + W, at, ah, :]
                            nc.scalar.copy(out=dst, in_=src)
                    nc.sync.dma_start(
                        out=Lt[p][s * SCH : s * SCH + live], in_=st[:live])

                Lview = Lt[p].rearrange("r (u e) -> (r u) e", e=4 * C)
                # ---- quads: fused weight/index math + per-chunk gather ----
                for q in range(NQ):
                    TQ = q * QC
                    n0q = TQ * CHUNK
                    D4 = wq.tile([128, 3, QC, M], F32, tag="D4")
                    for a in range(3):
                        nc.sync.dma_start(
                            out=D4[:, a],
                            in_=offs[p][a, n0q : n0q + QC * CHUNK].rearrange(
                                "(j p m) -> p j m", j=QC, m=M))
                    G2 = D4
                    nc.vector.tensor_add(out=G2, in0=D4, in1=g0)
                    i0 = wq.tile([128, 3, QC, M], F32, tag="i0")
                    nc.vector.tensor_scalar(out=i0, in0=G2, scalar1=MAGIC,
                                            scalar2=MAGIC, op0=OP.add,
                                            op1=OP.subtract)
                    f = wq.tile([128, 3, QC, M], F32, tag="f")
                    nc.vector.tensor_tensor(out=f, in0=i0, in1=G2, op=OP.is_gt)
                    nc.vector.tensor_sub(out=i0, in0=i0, in1=f)
                    nc.vector.tensor_sub(out=f, in0=G2, in1=i0)
                    w0 = wq.tile([128, 3, QC, M], F32, tag="w0")
                    nc.vector.tensor_scalar(out=w0, in0=f, scalar1=-1.0,
                                            scalar2=1.0, op0=OP.mult,
                                            op1=OP.add)
                    v = wq.tile([128, 3, QC, M], F32, tag="v")
                    nc.vector.tensor_scalar(out=v[:, 0], in0=i0[:, 0],
                                            scalar1=float(-TQ), scalar2=None,
                                            op0=OP.is_ge)
                    nc.vector.tensor_scalar(out=v[:, 1:3], in0=i0[:, 1:3],
                                            scalar1=0.0, scalar2=None,
                                            op0=OP.is_ge)
                    nc.vector.tensor_mul(out=w0, in0=w0, in1=v)
                    nc.vector.tensor_scalar(out=v[:, 0], in0=i0[:, 0],
                                            scalar1=float(T - 1 - TQ),
                                            scalar2=None, op0=OP.is_le)
                    nc.vector.tensor_scalar(out=v[:, 1:3], in0=i0[:, 1:3],
                                            scalar1=float(H - 1), scalar2=None,
                                            op0=OP.is_le)
                    nc.vector.tensor_mul(out=w0, in0=w0, in1=v)
                    w1 = f
                    nc.vector.tensor_scalar(out=v[:, 0], in0=i0[:, 0],
                                            scalar1=float(-1 - TQ),
                                            scalar2=None, op0=OP.is_ge)
                    nc.vector.tensor_scalar(out=v[:, 1:3], in0=i0[:, 1:3],
                                            scalar1=-1.0, scalar2=None,
                                            op0=OP.is_ge)
                    nc.vector.tensor_mul(out=w1, in0=f, in1=v)
                    nc.vector.tensor_scalar(out=v[:, 0], in0=i0[:, 0],
                                            scalar1=float(T - 2 - TQ),
                                            scalar2=None, op0=OP.is_le)
                    nc.vector.tensor_scalar(out=v[:, 1:3], in0=i0[:, 1:3],
                                            scalar1=float(H - 2), scalar2=None,
                                            op0=OP.is_le)
                    nc.vector.tensor_mul(out=w1, in0=w1, in1=v)
                    ic = i0
                    nc.vector.tensor_scalar_max(out=ic[:, 0], in0=i0[:, 0],
                                                scalar1=float(-1 - TQ))
                    nc.vector.tensor_scalar_max(out=ic[:, 1:3], in0=i0[:, 1:3],
                                                scalar1=-1.0)
                    nc.vector.tensor_scalar(out=ic[:, 0], in0=ic[:, 0],
                                            scalar1=float(T - 1 - TQ),
                                            scalar2=float(TQ + 1),
                                            op0=OP.min, op1=OP.add)
                    nc.vector.tensor_scalar(out=ic[:, 1:3], in0=ic[:, 1:3],
                                            scalar1=float(H - 1), scalar2=1.0,
                                            op0=OP.min, op1=OP.add)
                    idxf = wq.tile([128, QC, M], F32, tag="idxf")
                    nc.vector.tensor_scalar_mul(out=idxf, in0=ic[:, 0],
                                                scalar1=float(HB))
                    nc.vector.tensor_add(out=idxf, in0=idxf, in1=ic[:, 1])
                    nc.vector.tensor_scalar_mul(out=idxf, in0=idxf,
                                                scalar1=float(WW))
                    nc.vector.tensor_add(out=idxf, in0=idxf, in1=ic[:, 2])
                    idxu = wq.tile([128, QC, M], U32, tag="idxu")
                    nc.vector.tensor_copy(out=idxu, in_=idxf)
                    if dbg and p == 0 and q == 0:
                        nc.sync.dma_start(
                            out=dbg_idx,
                            in_=idxu.rearrange("p j m -> p (j m)"))
                        nc.sync.dma_start(out=dbg_l, in_=Lt[p][:128, :512])

                    for j in range(QC):
                        n0 = (TQ + j) * CHUNK
                        tmp = wts.tile([128, M, 2, 2], F32, tag="tmp")
                        for at in range(2):
                            wt_ = (w0 if at == 0 else w1)[:, 0, j]
                            for ah in range(2):
                                wh_ = (w0 if ah == 0 else w1)[:, 1, j]
                                nc.vector.tensor_mul(out=tmp[:, :, at, ah],
                                                     in0=wt_, in1=wh_)
                        W8 = wts.tile([128, M, 2, 2, 2], F32, tag="W8")
                        for aw in range(2):
                            ww_ = (w0 if aw == 0 else w1)[:, 2, j]
                            nc.vector.tensor_mul(
                                out=W8[:, :, aw],
                                in0=tmp,
                                in1=ww_.unsqueeze(2).unsqueeze(3).broadcast_to(
                                    (128, M, 2, 2)),
                            )
                        G = gbig.tile([128, M, 64], F16, tag="G")
                        for m in range(M):
                            nc.gpsimd.indirect_dma_start(
                                out=G[:, m],
                                out_offset=None,
                                in_=Lview,
                                in_offset=IndirectOffsetOnAxis(
                                    ap=idxu[:, j, m : m + 1], axis=0),
                            )
                        if dbg and p == 0 and q == 0 and j == 0:
                            nc.sync.dma_start(
                                out=dbg_g.rearrange("p (m e) -> p m e", e=64),
                                in_=G)
                            nc.sync.dma_start(
                                out=dbg_w8,
                                in_=W8.rearrange("p m a b c -> p (m a b c)"))
                        Gv = G.rearrange("p m (k c) -> p m k c", k=8)
                        nc.vector.tensor_mul(
                            out=Gv, in0=Gv,
                            in1=W8.rearrange("p m a b c -> p m (a b c)")
                            .unsqueeze(3).broadcast_to((128, M, 8, C)),
                        )
                        Ge = G
                        nc.vector.tensor_add(out=Ge[:, :, 0:32],
                                             in0=Ge[:, :, 0:32],
                                             in1=Ge[:, :, 32:64])
                        nc.vector.tensor_add(out=Ge[:, :, 0:16],
                                             in0=Ge[:, :, 0:16],
                                             in1=Ge[:, :, 16:32])
                        OC = ocp.tile([128, C, M], F32, tag="OC")
                        nc.vector.tensor_add(
                            out=OC.rearrange("p c m -> p m c"),
                            in0=Ge[:, :, 0:8], in1=Ge[:, :, 8:16])
                        nc.sync.dma_start(
                            out=outs[p][:, n0 : n0 + CHUNK].rearrange(
                                "c (p m) -> p c m", m=M),
                            in_=OC,
                        )
    nc.compile()
    return nc


def shard_inputs(cfg: Cfg, x: np.ndarray, offset: np.ndarray, n_cores: int = 8):
    B, Ct, T, H, W = x.shape
    DG = offset.shape[1] // 3
    Cg = Ct // DG
    NP = cfg.NPAIRS
    grids = np.ascontiguousarray(
        make_grids(cfg)[:, : cfg.QC * cfg.CHUNK], dtype=np.float32)
    # [B, DG, T+2, H+3, W, Cg] f16, zero-padded t:(1,1), h:(1,2)
    xall = np.pad(x.reshape(B, DG, Cg, T, H, W),
                  ((0, 0), (0, 0), (0, 0), (1, 1), (1, 2), (0, 0)))
    xall = np.ascontiguousarray(
        xall.transpose(0, 1, 3, 4, 5, 2), dtype=np.float16)
    offr = offset.reshape(B, DG, 3, cfg.N)
    in_maps = []
    for core in range(n_cores):
        xsv = np.empty((NP, T + 2, H + 3, W, Cg), np.float16)
        offv = np.empty((NP, 3, cfg.N), np.float32)
        for i in range(NP):
            flat = core * NP + i
            b, g = flat // DG, flat % DG
            xsv[i] = xall[b, g]
            offv[i] = offr[b, g]
        in_maps.append({"xs": xsv, "offs": offv, "grids": grids})
    return in_maps


def unshard_outputs(cfg: Cfg, results, B, Ct, n_cores: int = 8):
    DG = (n_cores * cfg.NPAIRS) // B
    Cg = Ct // DG
    out = np.empty((B, Ct, cfg.T, cfg.H, cfg.W), np.float32)
    for core in range(n_cores):
        o = results[core]["outs"].reshape(cfg.NPAIRS, Cg, cfg.T, cfg.H, cfg.W)
        for i in range(cfg.NPAIRS):
            flat = core * cfg.NPAIRS + i
            b, g = flat // DG, flat % DG
            out[b, g * Cg : (g + 1) * Cg] = o[i]
    return out


_CACHE = {}


def _get_program(cfg):
    if "nc" not in _CACHE:
        _CACHE["nc"] = build_program(cfg)
    return _CACHE["nc"]


def kernel(x, offset, weight=None, bias=None, **_):
    """Deformable 3D sampling with identity 1x1x1 conv (weight=I, bias=0).

    x: [4, 64, 8, 128, 128] f32; offset: [4, 24, 8, 128, 128] f32.
    Returns [4, 64, 8, 128, 128] f32. Runs on 8 NeuronCores, data-parallel
    over the 32 (batch, deformable-group) pairs (4 pairs per core).
    """
    import concourse.bass_utils as bass_utils

    x = np.ascontiguousarray(np.asarray(x, dtype=np.float32))
    offset = np.ascontiguousarray(np.asarray(offset, dtype=np.float32))
    B, Ct = x.shape[0], x.shape[1]
    cfg = Cfg()
    nc = _get_program(cfg)
    in_maps = shard_inputs(cfg, x, offset, n_cores=8)
    res = bass_utils.run_bass_kernel_spmd(nc, in_maps, core_ids=list(range(8)))
    return unshard_outputs(cfg, res.results, B, Ct, n_cores=8)

